# revision 3
# baseline (speedup 1.0000x reference)
"""ALSH Conv2d kernel for 8 Trainium2 NeuronCores.

Strategy (data-parallel, host-routed sparsity):
  - The reference output is `dense_conv(x, W) * active_mask` where
    active[n, o] = (bucket(kernel_o) == bucket(sample_n)).  Masked-out
    channels are exactly zero, so we only compute the active channels.
  - Host computes the ALSH buckets in float64 (hash dots sit far from
    integer floor boundaries relative to f32 noise, so this matches the
    f32 reference bit-exactly), gathers each sample's active kernel rows,
    and packs them into <=128-channel "atoms".
  - 8 cores, 4 samples each, fixed atoms/sample-slot layout so all cores
    run one SPMD graph. Each atom runs a 3x3 conv as 18 accumulating
    matmuls (9 taps x 2 C-chunks of 128) over 7 pixel tiles of 448.
  - Host scatters atom outputs back into the zero-initialized full
    [32, 512, 56, 56] output.
"""

import os
import sys
import types

import numpy as np

N, C, H, W = 32, 256, 56, 56
O, KS = 512, 3
D = KS * KS * C  # 2304
TABLE_SIZE = 16
M_AUG = 3
NCORES = 8
S_PER_CORE = N // NCORES  # 4
HP, WP = H + 2, W + 2  # 58 padded
TILE_ROWS = 8
NT = H // TILE_ROWS  # 7 pixel tiles
TILE_PIX = TILE_ROWS * W  # 448
NJ = 2 * KS * KS  # 18 contraction chunks
CCH = C // 128  # 2 chunks of C

COMPUTE_DTYPE = os.environ.get("ALSH_DTYPE", "f32r")  # f32 | f32r | bf16

_graph_cache = {}
last_exec_time_ns = None
last_results = None


def _install_patches():
    """Walrus in this container encodes at most 1 sync wait per CTRL
    instruction; Tile's kernel-tail drain can carry several. Split them
    across consecutive drains (same engine => same ordering semantics)."""
    import concourse.tile as tile
    import concourse.mybir as mybir
    from concourse.vector_clock import ScopedClock

    if getattr(tile.TileContext, "_alsh_patched", False):
        return

    def _patched(self, tick_clock, wait_clock):
        nc = self.nc
        drain_inst = nc.sync.drain()
        wait_clock.add_sem_waits(
            drain_inst.ins, ScopedClock({None: tick_clock.global_clock})
        )
        si = drain_inst.ins.sync_info
        waits = list(si.on_wait or []) if si is not None else []
        if len(waits) > 1:
            si.on_wait = waits[:1]
            for i in range(1, len(waits)):
                d2 = nc.sync.drain()
                if d2.ins.sync_info is None:
                    d2.ins.sync_info = mybir.SyncInfo(
                        on_wait=waits[i : i + 1], on_update=[]
                    )
                else:
                    d2.ins.sync_info.on_wait = waits[i : i + 1]
        nc.all_engine_barrier()
        assert self.sems is not None
        popped = nc._tile_sem_poison_stack.pop()
        assert popped is self._sem_poison
        nc.clear_and_free_semaphores(list(self.sems.allocated().values()))
        nc.all_engine_barrier()

    tile.TileContext._drain_and_barrier = _patched
    tile.TileContext._alsh_patched = True


def _split_excess_waits(nc, max_waits=1):
    """Walrus here encodes at most one sync wait per instruction. Hoist
    excess waits onto no-op carrier instructions inserted immediately
    before the overloaded instruction on the same engine (engines run
    their block instructions in order, so this is semantics-preserving)."""
    import bass_rust
    import concourse.mybir as mybir

    ctr = [0]

    def carrier(engine, waits):
        ctr[0] += 1
        nop = bass_rust.InstNoOp(name=f"WSPLIT-{ctr[0]}", engine=engine)
        nop.sync_info = mybir.SyncInfo(on_wait=list(waits), on_update=[])
        return nop

    n_split = 0
    for fn in nc.m.functions:
        for bb in fn.blocks:
            out = []
            for inst in bb.instructions:
                si = inst.sync_info
                if si is not None and si.on_wait and len(si.on_wait) > max_waits:
                    waits = list(si.on_wait)
                    si.on_wait = waits[-max_waits:]
                    extra = waits[: -max_waits]
                    for i in range(0, len(extra), max_waits):
                        out.append(carrier(inst.engine, extra[i : i + max_waits]))
                    n_split += 1
                out.append(inst)
            bb.instructions[:] = out
    return n_split


def _install_trace_hook():
    try:
        from antenv import axon_hooks  # noqa: F401
        return
    except ImportError:
        pass
    try:
        from trn_agent_boot.trn_boot import _ntff_profile_via_ctypes
    except ImportError:
        return
    hook = _ntff_profile_via_ctypes("/opt/axon/libaxon_pjrt.so")
    m = types.ModuleType("antenv.axon_hooks")
    m.get_axon_ntff_profile_hook = lambda: hook
    m.set_axon_ntff_profile_hook = lambda h: None
    sys.modules["antenv.axon_hooks"] = m
    import antenv

    antenv.axon_hooks = m


def _bucket64(dots):
    return np.mod(np.abs(np.floor(dots)), TABLE_SIZE).astype(np.int32)


def _routing(x, kernels, hash_a):
    """Replicate the reference hashing in float64 on host."""
    a_main = hash_a[:D].astype(np.float64)
    a_aug = hash_a[D:].astype(np.float64)
    k64 = kernels.astype(np.float64)
    n2 = np.sum(k64 * k64, axis=1)
    powers = np.stack([n2 ** (2 ** i) for i in range(M_AUG)], axis=1)
    k_dots = k64 @ a_main + powers @ a_aug
    k_bucket = _bucket64(k_dots)

    q = x.astype(np.float64).mean(axis=(2, 3))  # [N, C]
    q_t = np.tile(q, (1, KS * KS))  # [N, D]
    q_dots = q_t @ a_main + 0.5 * np.sum(a_aug)
    q_bucket = _bucket64(q_dots)
    return k_bucket, q_bucket


def _mybir_dtype(mybir):
    return {
        "f32": mybir.dt.float32,
        "f32r": mybir.dt.float32r,
        "bf16": mybir.dt.bfloat16,
    }[COMPUTE_DTYPE]


def _np_in_dtype():
    if COMPUTE_DTYPE == "bf16":
        import ml_dtypes

        return ml_dtypes.bfloat16
    return np.float32


def _build_graph(a_slots):
    """Build the SPMD Bass graph for one core. a_slots[s] = number of
    weight atoms processed against local sample s."""
    import concourse.bass as bass
    import concourse.mybir as mybir
    import concourse.tile as tile

    A = sum(a_slots)
    dt_in = _mybir_dtype(mybir)
    f32 = mybir.dt.float32

    nc = bass.Bass()
    xs_ext = nc.declare_dram_parameter(
        "xs", [S_PER_CORE, CCH, 128, HP, WP], dt_in, isOutput=False
    )
    ws_ext = nc.declare_dram_parameter(
        "ws", [A, 128, NJ, 128], dt_in, isOutput=False
    )
    out_ext = nc.declare_dram_parameter("out", [A, 128, NT * TILE_PIX], f32, isOutput=True)

    with tile.TileContext(nc) as tc:
        with (
            tc.tile_pool(name="xp", bufs=2) as xpool,
            tc.tile_pool(name="wp", bufs=3) as wpool,
            tc.tile_pool(name="op", bufs=2) as opool,
            tc.tile_pool(name="pp", bufs=4, space="PSUM") as ppool,
        ):
            a = 0
            for s in range(S_PER_CORE):
                xt = []
                for c2 in range(CCH):
                    t = xpool.tile([128, HP, WP], dt_in, tag=f"x{c2}")
                    nc.sync.dma_start(t[:], xs_ext[s, c2])
                    xt.append(t)
                for _k in range(a_slots[s]):
                    wt = wpool.tile([128, NJ, 128], dt_in, tag="w")
                    nc.sync.dma_start(wt[:], ws_ext[a])
                    ot = opool.tile([128, NT, TILE_ROWS, W], f32, tag="o")
                    for t in range(NT):
                        r0 = t * TILE_ROWS
                        pt = ppool.tile([128, TILE_ROWS, W], f32, tag="ps")
                        j = 0
                        for kh in range(KS):
                            for kw in range(KS):
                                for c2 in range(CCH):
                                    rhs = xt[c2][
                                        :, r0 + kh : r0 + kh + TILE_ROWS, kw : kw + W
                                    ]
                                    nc.tensor.matmul(
                                        pt[:],
                                        wt[:, (kh * KS + kw) * CCH + c2, :],
                                        rhs,
                                        start=(j == 0),
                                        stop=(j == NJ - 1),
                                    )
                                    j += 1
                        nc.vector.tensor_copy(ot[:, t], pt[:])
                    nc.sync.dma_start(out_ext[a], ot[:].rearrange("p a b c -> p (a b c)"))
                    a += 1
    return nc


def kernel(x, kernels, hash_a, mode=None):
    x = np.ascontiguousarray(np.asarray(x, dtype=np.float32))
    kernels = np.ascontiguousarray(np.asarray(kernels, dtype=np.float32))
    hash_a = np.asarray(hash_a, dtype=np.float32)

    k_bucket, q_bucket = _routing(x, kernels, hash_a)

    # Per-sample active channel lists and atom counts.
    idx_lists = [np.where(k_bucket == q_bucket[n])[0] for n in range(N)]
    atoms_of = [int(-(-len(ix) // 128)) for ix in idx_lists]  # ceil

    # Assign samples to cores: sort by atom count desc, snake over cores.
    order = sorted(range(N), key=lambda n: (-atoms_of[n], n))
    core_samples = [[] for _ in range(NCORES)]
    for i, n in enumerate(order):
        blk, pos = divmod(i, NCORES)
        c = pos if blk % 2 == 0 else NCORES - 1 - pos
        core_samples[c].append(n)
    # Within each core sort samples desc by atoms so slot k has the max
    # atom count across cores (uniform static graph).
    for c in range(NCORES):
        core_samples[c].sort(key=lambda n: (-atoms_of[n], n))
    a_slots = [
        max(atoms_of[core_samples[c][s]] for c in range(NCORES))
        for s in range(S_PER_CORE)
    ]
    a_slots = [max(k, 1) for k in a_slots]
    A = sum(a_slots)

    out_full = np.zeros((N, O, H, W), dtype=np.float32)
    if all(len(ix) == 0 for ix in idx_lists):
        return out_full

    np_in = _np_in_dtype()
    # Pack per-core inputs.
    in_maps = []
    scatter = []  # per core: list of (atom_idx, sample, channel_indices)
    kern4 = kernels.reshape(O, KS * KS, CCH, 128)  # [o, tap, c2, c]
    for c in range(NCORES):
        xs = np.zeros((S_PER_CORE, CCH, 128, HP, WP), dtype=np_in)
        ws = np.zeros((A, 128, NJ, 128), dtype=np_in)
        sc = []
        a = 0
        for s in range(S_PER_CORE):
            n = core_samples[c][s]
            xs[s, :, :, 1 : H + 1, 1 : W + 1] = x[n].reshape(CCH, 128, H, W)
            ix = idx_lists[n]
            for k in range(a_slots[s]):
                chans = ix[k * 128 : (k + 1) * 128]
                if len(chans):
                    # [cnt, tap, c2, c] -> [c, tap*CCH, cnt]
                    blk = kern4[chans]
                    blk = blk.transpose(3, 1, 2, 0).reshape(128, NJ, len(chans))
                    ws[a, :, :, : len(chans)] = blk
                    sc.append((a, n, chans))
                a += 1
        in_maps.append({"xs": xs, "ws": ws})
        scatter.append(sc)

    # Build / fetch graph and run.
    _install_patches()
    key = tuple(a_slots) + (COMPUTE_DTYPE,)
    if key not in _graph_cache:
        nc_new = _build_graph(a_slots)
        _split_excess_waits(nc_new)
        _graph_cache[key] = nc_new
    nc = _graph_cache[key]

    trace = bool(int(os.environ.get("ALSH_TRACE", "0")))
    if trace:
        _install_trace_hook()
        import concourse.bass_utils as bu

        bu.upload_artifacts = lambda d: d

    from concourse.bass_utils import run_bass_kernel_spmd

    res = run_bass_kernel_spmd(
        nc, in_maps, list(range(NCORES)), trace=trace
    )
    global last_exec_time_ns, last_results
    last_exec_time_ns = res.exec_time_ns
    last_results = res

    for c in range(NCORES):
        out_c = res.results[c]["out"]  # [A, 128, 3136]
        for (a, n, chans) in scatter[c]:
            out_full[n, chans] = out_c[a, : len(chans)].reshape(len(chans), H, W)
    return out_full


# revision 8
# speedup vs baseline: 1.0121x; 1.0121x over previous
"""ALSH Conv2d kernel for 8 Trainium2 NeuronCores.

Strategy (data-parallel, host-routed sparsity):
  - The reference output is `dense_conv(x, W) * active_mask` where
    active[n, o] = (bucket(kernel_o) == bucket(sample_n)).  Masked-out
    channels are exactly zero, so we only compute the active channels.
  - Host computes the ALSH buckets in float64 (hash dots sit far from
    integer floor boundaries relative to f32 noise, so this matches the
    f32 reference bit-exactly), gathers each sample's active kernel rows,
    and packs them into <=128-channel "atoms".
  - 8 cores, 4 samples each, fixed atoms/sample-slot layout so all cores
    run one SPMD graph. Each atom runs a 3x3 conv as 18 accumulating
    matmuls (9 taps x 2 C-chunks of 128) over 7 pixel tiles of 448.
  - Host scatters atom outputs back into the zero-initialized full
    [32, 512, 56, 56] output.
"""

import os
import sys
import types

import numpy as np

N, C, H, W = 32, 256, 56, 56
O, KS = 512, 3
D = KS * KS * C  # 2304
TABLE_SIZE = 16
M_AUG = 3
NCORES = 8
S_PER_CORE = N // NCORES  # 4
HP, WP = H + 2, W + 2  # 58 padded
TILE_ROWS = 8
NT = H // TILE_ROWS  # 7 pixel tiles
TILE_PIX = TILE_ROWS * W  # 448
NJ = 2 * KS * KS  # 18 contraction chunks
CCH = C // 128  # 2 chunks of C

COMPUTE_DTYPE = os.environ.get("ALSH_DTYPE", "bf16")  # f32 | f32r | bf16
OUT_DTYPE = os.environ.get("ALSH_OUT_DTYPE", "bf16")  # f32 | bf16
XROW_SPLITS = [0, 12, 35, HP]  # x DMA row blocks so tile 0 can start early

_graph_cache = {}
last_exec_time_ns = None
last_results = None


def _install_patches():
    """Walrus in this container encodes at most 1 sync wait per CTRL
    instruction; Tile's kernel-tail drain can carry several. Split them
    across consecutive drains (same engine => same ordering semantics)."""
    import concourse.tile as tile
    import concourse.mybir as mybir
    from concourse.vector_clock import ScopedClock

    if getattr(tile.TileContext, "_alsh_patched", False):
        return

    def _patched(self, tick_clock, wait_clock):
        nc = self.nc
        drain_inst = nc.sync.drain()
        wait_clock.add_sem_waits(
            drain_inst.ins, ScopedClock({None: tick_clock.global_clock})
        )
        si = drain_inst.ins.sync_info
        waits = list(si.on_wait or []) if si is not None else []
        if len(waits) > 1:
            si.on_wait = waits[:1]
            for i in range(1, len(waits)):
                d2 = nc.sync.drain()
                if d2.ins.sync_info is None:
                    d2.ins.sync_info = mybir.SyncInfo(
                        on_wait=waits[i : i + 1], on_update=[]
                    )
                else:
                    d2.ins.sync_info.on_wait = waits[i : i + 1]
        nc.all_engine_barrier()
        assert self.sems is not None
        popped = nc._tile_sem_poison_stack.pop()
        assert popped is self._sem_poison
        nc.clear_and_free_semaphores(list(self.sems.allocated().values()))
        nc.all_engine_barrier()

    tile.TileContext._drain_and_barrier = _patched
    tile.TileContext._alsh_patched = True


def _split_excess_waits(nc, max_waits=1):
    """Walrus here encodes at most one sync wait per instruction. Hoist
    excess waits onto no-op carrier instructions inserted immediately
    before the overloaded instruction on the same engine (engines run
    their block instructions in order, so this is semantics-preserving)."""
    import bass_rust
    import concourse.mybir as mybir

    ctr = [0]

    def carrier(engine, waits):
        ctr[0] += 1
        nop = bass_rust.InstNoOp(name=f"WSPLIT-{ctr[0]}", engine=engine)
        nop.sync_info = mybir.SyncInfo(on_wait=list(waits), on_update=[])
        return nop

    n_split = 0
    for fn in nc.m.functions:
        for bb in fn.blocks:
            out = []
            for inst in bb.instructions:
                si = inst.sync_info
                if si is not None and si.on_wait and len(si.on_wait) > max_waits:
                    waits = list(si.on_wait)
                    si.on_wait = waits[-max_waits:]
                    extra = waits[: -max_waits]
                    for i in range(0, len(extra), max_waits):
                        out.append(carrier(inst.engine, extra[i : i + max_waits]))
                    n_split += 1
                out.append(inst)
            bb.instructions[:] = out
    return n_split


def _install_trace_hook():
    try:
        from antenv import axon_hooks  # noqa: F401
        return
    except ImportError:
        pass
    try:
        from trn_agent_boot.trn_boot import _ntff_profile_via_ctypes
    except ImportError:
        return
    hook = _ntff_profile_via_ctypes("/opt/axon/libaxon_pjrt.so")
    m = types.ModuleType("antenv.axon_hooks")
    m.get_axon_ntff_profile_hook = lambda: hook
    m.set_axon_ntff_profile_hook = lambda h: None
    sys.modules["antenv.axon_hooks"] = m
    import antenv

    antenv.axon_hooks = m


def _bucket64(dots):
    return np.mod(np.abs(np.floor(dots)), TABLE_SIZE).astype(np.int32)


def _routing(x, kernels, hash_a):
    """Replicate the reference hashing in float64 on host."""
    a_main = hash_a[:D].astype(np.float64)
    a_aug = hash_a[D:].astype(np.float64)
    k64 = kernels.astype(np.float64)
    n2 = np.sum(k64 * k64, axis=1)
    powers = np.stack([n2 ** (2 ** i) for i in range(M_AUG)], axis=1)
    k_dots = k64 @ a_main + powers @ a_aug
    k_bucket = _bucket64(k_dots)

    q = x.astype(np.float64).mean(axis=(2, 3))  # [N, C]
    q_t = np.tile(q, (1, KS * KS))  # [N, D]
    q_dots = q_t @ a_main + 0.5 * np.sum(a_aug)
    q_bucket = _bucket64(q_dots)
    return k_bucket, q_bucket


def _mybir_dtype(mybir):
    return {
        "f32": mybir.dt.float32,
        "f32r": mybir.dt.float32r,
        "bf16": mybir.dt.bfloat16,
    }[COMPUTE_DTYPE]


def _np_in_dtype():
    if COMPUTE_DTYPE == "bf16":
        import ml_dtypes

        return ml_dtypes.bfloat16
    return np.float32


def _build_graph(a_slots):
    """Build the SPMD Bass graph for one core. a_slots[s] = number of
    weight atoms processed against local sample s."""
    import concourse.bass as bass
    import concourse.mybir as mybir
    import concourse.tile as tile

    A = sum(a_slots)
    dt_in = _mybir_dtype(mybir)
    f32 = mybir.dt.float32
    dt_out = mybir.dt.bfloat16 if OUT_DTYPE == "bf16" else f32

    nc = bass.Bass()
    xs_ext = nc.declare_dram_parameter(
        "xs", [S_PER_CORE, CCH, 128, HP, WP], dt_in, isOutput=False
    )
    ws_ext = nc.declare_dram_parameter(
        "ws", [A, 128, NJ, 128], dt_in, isOutput=False
    )
    out_ext = nc.declare_dram_parameter(
        "out", [A, 128, NT * TILE_PIX], dt_out, isOutput=True
    )

    with tile.TileContext(nc) as tc:
        with (
            tc.tile_pool(name="xp", bufs=2) as xpool,
            tc.tile_pool(name="wp", bufs=3) as wpool,
            tc.tile_pool(name="op", bufs=2) as opool,
            tc.tile_pool(name="pp", bufs=4, space="PSUM") as ppool,
        ):
            a = 0
            for s in range(S_PER_CORE):
                xt = [
                    xpool.tile(
                        [128, HP, WP], dt_in, tag=f"x{c2}", name=f"xt{c2}"
                    )
                    for c2 in range(CCH)
                ]
                # Row-blocked loads so tile 0's matmuls start after a small
                # fraction of x has landed (matters for the first sample).
                for r0, r1 in zip(XROW_SPLITS, XROW_SPLITS[1:]):
                    for c2 in range(CCH):
                        nc.sync.dma_start(xt[c2][:, r0:r1], xs_ext[s, c2, :, r0:r1])
                for _k in range(a_slots[s]):
                    wt = wpool.tile([128, NJ, 128], dt_in, tag="w")
                    # split so the c2=0 half (needed first) arrives first
                    nc.sync.dma_start(wt[:, : NJ // 2], ws_ext[a, :, : NJ // 2])
                    nc.sync.dma_start(wt[:, NJ // 2 :], ws_ext[a, :, NJ // 2 :])
                    ot = opool.tile([128, NT, TILE_ROWS, W], dt_out, tag="o")
                    for t in range(NT):
                        r0 = t * TILE_ROWS
                        pt = ppool.tile([128, TILE_ROWS, W], f32, tag="ps")
                        j = 0
                        for c2 in range(CCH):
                            for kh in range(KS):
                                for kw in range(KS):
                                    rhs = xt[c2][
                                        :, r0 + kh : r0 + kh + TILE_ROWS, kw : kw + W
                                    ]
                                    nc.tensor.matmul(
                                        pt[:],
                                        wt[:, c2 * KS * KS + kh * KS + kw, :],
                                        rhs,
                                        start=(j == 0),
                                        stop=(j == NJ - 1),
                                    )
                                    j += 1
                        nc.vector.tensor_copy(ot[:, t], pt[:])
                    nc.sync.dma_start(out_ext[a], ot[:].rearrange("p a b c -> p (a b c)"))
                    a += 1
    return nc


def kernel(x, kernels, hash_a, mode=None):
    x = np.ascontiguousarray(np.asarray(x, dtype=np.float32))
    kernels = np.ascontiguousarray(np.asarray(kernels, dtype=np.float32))
    hash_a = np.asarray(hash_a, dtype=np.float32)

    k_bucket, q_bucket = _routing(x, kernels, hash_a)

    # Per-sample active channel lists and atom counts.
    idx_lists = [np.where(k_bucket == q_bucket[n])[0] for n in range(N)]
    atoms_of = [int(-(-len(ix) // 128)) for ix in idx_lists]  # ceil

    # Assign samples to cores: sort by atom count desc, snake over cores.
    order = sorted(range(N), key=lambda n: (-atoms_of[n], n))
    core_samples = [[] for _ in range(NCORES)]
    for i, n in enumerate(order):
        blk, pos = divmod(i, NCORES)
        c = pos if blk % 2 == 0 else NCORES - 1 - pos
        core_samples[c].append(n)
    # Within each core sort samples desc by atoms so slot k has the max
    # atom count across cores (uniform static graph).
    for c in range(NCORES):
        core_samples[c].sort(key=lambda n: (-atoms_of[n], n))
    a_slots = [
        max(atoms_of[core_samples[c][s]] for c in range(NCORES))
        for s in range(S_PER_CORE)
    ]
    a_slots = [max(k, 1) for k in a_slots]
    A = sum(a_slots)

    out_full = np.zeros((N, O, H, W), dtype=np.float32)
    if all(len(ix) == 0 for ix in idx_lists):
        return out_full

    np_in = _np_in_dtype()
    # Pack per-core inputs.
    in_maps = []
    scatter = []  # per core: list of (atom_idx, sample, channel_indices)
    kern4 = kernels.reshape(O, KS * KS, CCH, 128)  # [o, tap, c2, c]
    for c in range(NCORES):
        xs = np.zeros((S_PER_CORE, CCH, 128, HP, WP), dtype=np_in)
        ws = np.zeros((A, 128, NJ, 128), dtype=np_in)
        sc = []
        a = 0
        for s in range(S_PER_CORE):
            n = core_samples[c][s]
            xs[s, :, :, 1 : H + 1, 1 : W + 1] = x[n].reshape(CCH, 128, H, W)
            ix = idx_lists[n]
            for k in range(a_slots[s]):
                chans = ix[k * 128 : (k + 1) * 128]
                if len(chans):
                    # [cnt, tap, c2, c] -> [c, c2*9+tap, cnt]
                    blk = kern4[chans]
                    blk = blk.transpose(3, 2, 1, 0).reshape(128, NJ, len(chans))
                    ws[a, :, :, : len(chans)] = blk
                    sc.append((a, n, chans))
                a += 1
        in_maps.append({"xs": xs, "ws": ws})
        scatter.append(sc)

    # Build / fetch graph and run.
    _install_patches()
    key = tuple(a_slots) + (COMPUTE_DTYPE,)
    if key not in _graph_cache:
        nc_new = _build_graph(a_slots)
        _split_excess_waits(nc_new)
        _graph_cache[key] = nc_new
    nc = _graph_cache[key]

    trace = bool(int(os.environ.get("ALSH_TRACE", "0")))
    if trace:
        _install_trace_hook()
        import concourse.bass_utils as bu

        bu.upload_artifacts = lambda d: d

    from concourse.bass_utils import run_bass_kernel_spmd

    res = run_bass_kernel_spmd(
        nc, in_maps, list(range(NCORES)), trace=trace
    )
    global last_exec_time_ns, last_results
    last_exec_time_ns = res.exec_time_ns
    last_results = res

    for c in range(NCORES):
        out_c = np.asarray(res.results[c]["out"], dtype=np.float32)  # [A, 128, 3136]
        for (a, n, chans) in scatter[c]:
            out_full[n, chans] = out_c[a, : len(chans)].reshape(len(chans), H, W)
    return out_full
